# revision 6
# baseline (speedup 1.0000x reference)
"""Trainium2 Bass kernel: multi-head attention (B=2, S=2048, E=1024, H=16).

Sharding: 8 cores = 2 batches x 4 head-groups. Core c handles batch c//4 and
heads [4*(c%4), 4*(c%4)+4) (256 feature columns of the projections).

Per-core device program (all matmuls in fp32r):
  - inputs: xT [E,S] (host-transposed x[b]), wqT/wkT/wvT [E,256] (host-
    transposed row-slices of Wq/Wk/Wv), woT [256,E] (host-transposed column
    slice of Wo).
  - qT,kT [256,S] = (x @ W^T)^T per head-group, computed directly in [f,s]
    layout; v [S,256] in [s,f] layout with a ones column appended per head.
  - per (head, qi-chunk): scores^T tiles [128 kj, 512 qi] on PE, exp on ACT
    (sm_scale folded into the activation scale), attn@v accumulated on PE with
    the ones column producing the softmax denominator in partition 64,
    reciprocal + broadcast-matmul + multiply to normalize; output kept in
    [f, s] layout for the output projection.
  - out_partial [S,E] = o^T^T @ Wo^T column-slice; host sums 4 partials per
    batch and adds bo.
"""

import numpy as np

import concourse.bass as bass
import concourse.tile as tile
import concourse.mybir as mybir
from concourse import bacc
from concourse.bass_utils import run_bass_kernel_spmd

B, S, E, H, D = 2, 2048, 1024, 16, 64
NCORES = 8
GPB = NCORES // B      # head-groups (cores) per batch = 4
HPC = H // GPB         # heads per core = 4
FPC = HPC * D          # feature cols per core = 256
SM = float(D) ** -0.5  # softmax scale

F32 = mybir.dt.float32
F32R = mybir.dt.float32r

P = 128
NE = E // P            # 8 e-tiles
NST = S // P           # 16 s-tiles (key tiles)
NQ = 4                 # qi chunks
QC = S // NQ           # 512
KTG = 2                # k-tiles per psum/exp group
NKG = NST // KTG       # 8 groups
FT = FPC // P          # 2 f-tiles per core


def _round_fp32r(a: np.ndarray) -> np.ndarray:
    """Round fp32 to the fp32r encoding (RNE to 12-bit mantissa)."""
    u = np.ascontiguousarray(a, dtype=np.float32).view(np.uint32)
    lo = u & np.uint32(0xFFF)
    base = u & ~np.uint32(0xFFF)
    rup = (lo > 0x800) | ((lo == 0x800) & (((base >> np.uint32(12)) & np.uint32(1)) == 1))
    out = base + (rup.astype(np.uint32) << np.uint32(12))
    return out.view(np.float32)


def _build():
    nc = bacc.Bacc("TRN2", target_bir_lowering=False, debug=False)

    xT_d = nc.dram_tensor("xT", [E, S], F32R, kind="ExternalInput")
    wq_d = nc.dram_tensor("wqT", [E, FPC], F32R, kind="ExternalInput")
    wk_d = nc.dram_tensor("wkT", [E, FPC], F32R, kind="ExternalInput")
    wv_d = nc.dram_tensor("wvT", [E, FPC], F32R, kind="ExternalInput")
    wo_d = nc.dram_tensor("woT", [FPC, E], F32R, kind="ExternalInput")
    ones_lhs_d = nc.dram_tensor("ones_lhs", [1, D], F32R, kind="ExternalInput")
    ones_col_d = nc.dram_tensor("ones_col", [P, HPC, 1], F32R, kind="ExternalInput")
    out_d = nc.dram_tensor("out", [S, E], F32, kind="ExternalOutput")

    with tile.TileContext(nc) as tc:
        with (
            tc.tile_pool(name="wpool", bufs=1) as wpool,
            tc.tile_pool(name="xpool", bufs=1) as xpool,
            tc.tile_pool(name="qkpool", bufs=1) as qkpool,
            tc.tile_pool(name="vpool", bufs=1) as vpool,
            tc.tile_pool(name="opool", bufs=1) as opool,
            tc.tile_pool(name="epool", bufs=3) as epool,
            tc.tile_pool(name="spool", bufs=2) as spool,
            tc.tile_pool(name="outpool", bufs=3) as outpool,
            tc.tile_pool(name="pspool", bufs=2, space="PSUM") as pspool,
            tc.tile_pool(name="popool", bufs=4, space="PSUM") as popool,
        ):
            # ---- weights / constants -------------------------------------
            wq = wpool.tile([P, NE, FPC], F32R, name="wq")
            wk = wpool.tile([P, NE, FPC], F32R, name="wk")
            wv = wpool.tile([P, NE, FPC], F32R, name="wv")
            wo = wpool.tile([P, FT, E], F32R, name="wo")
            nc.sync.dma_start(out=wq, in_=wq_d.ap().rearrange("(t p) f -> p t f", p=P))
            nc.sync.dma_start(out=wk, in_=wk_d.ap().rearrange("(t p) f -> p t f", p=P))
            nc.sync.dma_start(out=wv, in_=wv_d.ap().rearrange("(t p) f -> p t f", p=P))
            nc.sync.dma_start(out=wo, in_=wo_d.ap().rearrange("(t p) g -> p t g", p=P))
            ones = wpool.tile([1, D], F32R, name="ones")
            nc.sync.dma_start(out=ones, in_=ones_lhs_d.ap())

            # ---- x^T ------------------------------------------------------
            xts = []
            xT_r = xT_d.ap().rearrange("(t p) s -> p t s", p=P)
            for et in range(NE):
                xt = xpool.tile([P, S], F32R, name=f"xt{et}", tag=f"xt{et}")
                nc.sync.dma_start(out=xt, in_=xT_r[:, et, :])
                xts.append(xt)

            # ---- v projection: v[s, f] with ones col per head ------------
            v_tiles = []
            for st in range(NST):
                vt = vpool.tile([P, HPC, D + 1], F32R, name=f"v{st}", tag=f"v{st}")
                nc.sync.dma_start(out=vt[:, :, D : D + 1], in_=ones_col_d.ap())
                ps_v = popool.tile([P, FPC], F32, name="ps_v", tag="po")
                for et in range(NE):
                    nc.tensor.matmul(
                        ps_v,
                        xts[et][:, st * P : (st + 1) * P],
                        wv[:, et, :],
                        start=(et == 0),
                        stop=(et == NE - 1),
                    )
                nc.vector.tensor_copy(
                    vt[:, :, 0:D], ps_v.rearrange("p (h d) -> p h d", d=D)
                )
                v_tiles.append(vt)

            # ---- q^T / k^T projections: [f, s] ---------------------------
            def proj_T(w_tile, dst_tiles, which):
                for ft in range(FT):
                    for cq in range(NQ):
                        ps = popool.tile([P, QC], F32, name=f"ps_{which}", tag="po")
                        for et in range(NE):
                            nc.tensor.matmul(
                                ps,
                                w_tile[:, et, ft * P : (ft + 1) * P],
                                xts[et][:, cq * QC : (cq + 1) * QC],
                                start=(et == 0),
                                stop=(et == NE - 1),
                            )
                        nc.vector.tensor_copy(
                            dst_tiles[ft][:, cq * QC : (cq + 1) * QC], ps
                        )

            kts = [qkpool.tile([P, S], F32R, name=f"kt{ft}", tag=f"kt{ft}") for ft in range(FT)]
            qts = [qkpool.tile([P, S], F32R, name=f"qt{ft}", tag=f"qt{ft}") for ft in range(FT)]
            proj_T(wk, kts, "k")
            proj_T(wq, qts, "q")

            ots = [opool.tile([P, S], F32R, name=f"ot{ft}", tag=f"ot{ft}") for ft in range(FT)]

            # ---- attention + output projection ---------------------------
            for cq in range(NQ):
                for h in range(HPC):
                    ft, sub = h // 2, h % 2
                    lo, hi = sub * D, (sub + 1) * D
                    q_sl = qts[ft][lo:hi, cq * QC : (cq + 1) * QC]
                    ps_o = popool.tile([D + 1, QC], F32, name="ps_o", tag="po")
                    for g in range(NKG):
                        ps_s = pspool.tile([P, KTG, QC], F32, name="ps_s", tag="ps_s")
                        for j in range(KTG):
                            kt = g * KTG + j
                            nc.tensor.matmul(
                                ps_s[:, j, :],
                                kts[ft][lo:hi, kt * P : (kt + 1) * P],
                                q_sl,
                                start=True,
                                stop=True,
                            )
                        et_t = epool.tile([P, KTG, QC], F32R, name="et_t", tag="et_t")
                        nc.scalar.activation(
                            out=et_t,
                            in_=ps_s,
                            func=mybir.ActivationFunctionType.Exp,
                            scale=SM,
                        )
                        for j in range(KTG):
                            kt = g * KTG + j
                            nc.tensor.matmul(
                                ps_o,
                                v_tiles[kt][:, h, :],
                                et_t[:, j, :],
                                start=(kt == 0),
                                stop=(kt == NST - 1),
                            )
                    # softmax normalization: rows 0..63 are sum(exp*v),
                    # row 64 is sum(exp)
                    rec = spool.tile([1, QC], F32, name="rec", tag="rec")
                    nc.vector.reciprocal(rec, ps_o[D : D + 1, :])
                    rec_r = spool.tile([1, QC], F32R, name="rec_r", tag="rec_r")
                    nc.vector.tensor_copy(rec_r, rec)
                    ps_bc = popool.tile([D, QC], F32, name="ps_bc", tag="po")
                    nc.tensor.matmul(ps_bc, ones, rec_r, start=True, stop=True)
                    o_hat = epool.tile([D, QC], F32, name="o_hat", tag="o_hat")
                    nc.vector.tensor_copy(o_hat, ps_o[0:D, :])
                    nc.vector.tensor_mul(
                        ots[ft][lo:hi, cq * QC : (cq + 1) * QC], o_hat, ps_bc
                    )

                # output projection for the 4 s-tiles of this chunk
                for sti in range(NQ):
                    st = cq * NQ + sti
                    out_sb = outpool.tile([P, E], F32, name="out_sb", tag="out_sb")
                    for gc in range(2):
                        ps_out = popool.tile([P, QC], F32, name="ps_out", tag="po")
                        for ft in range(FT):
                            nc.tensor.matmul(
                                ps_out,
                                ots[ft][:, st * P : (st + 1) * P],
                                wo[:, ft, gc * QC : (gc + 1) * QC],
                                start=(ft == 0),
                                stop=(ft == FT - 1),
                            )
                        nc.vector.tensor_copy(
                            out_sb[:, gc * QC : (gc + 1) * QC], ps_out
                        )
                    nc.sync.dma_start(
                        out=out_d.ap()[st * P : (st + 1) * P, :], in_=out_sb
                    )

    nc.compile()
    return nc


_NC_CACHE = None


def _get_nc():
    global _NC_CACHE
    if _NC_CACHE is None:
        _NC_CACHE = _build()
    return _NC_CACHE


def make_in_maps(x, Wq, Wk, Wv, Wo):
    in_maps = []
    xTs = [_round_fp32r(x[b].T) for b in range(B)]
    for c in range(NCORES):
        b, hg = c // GPB, c % GPB
        fsl = slice(hg * FPC, (hg + 1) * FPC)
        in_maps.append({
            "xT": xTs[b],
            "wqT": _round_fp32r(Wq[fsl, :].T),
            "wkT": _round_fp32r(Wk[fsl, :].T),
            "wvT": _round_fp32r(Wv[fsl, :].T),
            "woT": _round_fp32r(Wo[:, fsl].T),
            "ones_lhs": np.ones((1, D), dtype=np.float32),
            "ones_col": np.ones((P, HPC, 1), dtype=np.float32),
        })
    return in_maps


def kernel(x, Wq, bq, Wk, bk, Wv, bv, Wo, bo):
    x = np.asarray(x, dtype=np.float32)
    Wq, Wk, Wv, Wo = (np.asarray(a, dtype=np.float32) for a in (Wq, Wk, Wv, Wo))
    bq, bk, bv, bo = (np.asarray(a, dtype=np.float32) for a in (bq, bk, bv, bo))
    if np.any(bq) or np.any(bk) or np.any(bv):
        # fall back: fold nonzero projection biases into an augmented input
        # row is not implemented; biases are zero for this problem spec.
        raise NotImplementedError("nonzero projection biases not supported")

    nc = _get_nc()
    in_maps = make_in_maps(x, Wq, Wk, Wv, Wo)
    res = run_bass_kernel_spmd(nc, in_maps, core_ids=list(range(NCORES)))
    out = np.empty((B, S, E), dtype=np.float32)
    for b in range(B):
        acc = res.results[b * GPB]["out"].astype(np.float32).copy()
        for hg in range(1, GPB):
            acc += res.results[b * GPB + hg]["out"]
        out[b] = acc
    out += bo[None, None, :]
    return out
